# revision 13
# baseline (speedup 1.0000x reference)
"""ComplexLayerScale Trainium2 kernel.

out[b,t,d] = (x_real + i*x_imag)[b,t,d] * (gamma_real + i*gamma_imag)[d]

Sharding: data-parallel over the batch dim (B=8 -> 8 NeuronCores), gamma
replicated. Per core: x shard [4096, 512] f32 per component; output stored
as interleaved (re, im) f32 pairs [4096, 1024] and viewed as complex64 on
the host (zero-copy).

Formulation (all DVE ops contiguous-output; stride-2 interleave writes
measured 2.8x slower, and GPSIMD/ACT cannot help - GPSIMD shares the DVE
read port pair and fully blocks during any 2-source DVE op, ACT only takes
per-partition scalars):
  G12 = [interleave(gr, gi) | interleave(-gi, gr)]   # host-built, O(D)
  xc  = [xr-rows | xi-rows]                          # one SBUF tile
  ab  = dup2(xc) * G12view    # ONE mul: A=xr*(gr,gi) pairs, B=xi*(-gi,gr)
  out = ab[:half] + ab[half:] # contiguous add, in place; pairs fall out
since out[2k] = xr*gr - xi*gi, out[2k+1] = xr*gi + xi*gr.

DVE work is read-port-bound at 6 cycles per complex element (the floor for
2-stream ops); everything else hides under it except the DMA head/tail.
Row chunks taper: 4x128 rows first (so the first mul starts as soon as
gamma + 512KB of x land), 6x512 in the middle, 2x256 at the end (so the
final store is 1 MiB). Loads+gamma on the sync HWDGE ring, stores on the
scalar ring.
"""

import numpy as np

# Problem shape (hardcoded per contract).
B, T, D = 8, 4096, 512
N_CORES = 8
P = 128                          # SBUF partitions
CHUNK_ROWS = [128] * 4 + [512] * 6 + [256] * 2   # sums to 4096

_CACHE = {}


def _build_program():
    import concourse.bacc as bacc
    import concourse.mybir as mybir
    import concourse.tile as tile

    f32 = mybir.dt.float32
    nc = bacc.Bacc("TRN2", target_bir_lowering=False, debug=False,
                   num_devices=N_CORES)

    xr = nc.dram_tensor("xr", [T, D], f32, kind="ExternalInput")
    xi = nc.dram_tensor("xi", [T, D], f32, kind="ExternalInput")
    g12 = nc.dram_tensor("g12", [P, 4 * D], f32, kind="ExternalInput")
    out = nc.dram_tensor("out", [T, 2 * D], f32, kind="ExternalOutput")

    with tile.TileContext(nc) as tc:
        with tc.tile_pool(name="gamma", bufs=1) as gpool, \
             tc.tile_pool(name="io", bufs=3) as iop, \
             tc.tile_pool(name="ab", bufs=3) as abp:

            # Host-replicated gamma pairs [P, 2*2D]: first transfer on the
            # sync ring so it lands with the first x chunk.
            gt = gpool.tile([P, 4 * D], f32, tag="gt")
            nc.sync.dma_start(out=gt[:], in_=g12[:])

            r0 = 0
            for rows in CHUNK_ROWS:
                rpp = rows // P          # rows per partition
                w = rpp * D              # x elems per partition per comp
                xc = iop.tile([P, 2 * w], f32, tag="xc")
                for half, src in ((0, xr), (1, xi)):
                    nc.sync.dma_start(
                        out=xc[:, half * w:(half + 1) * w],
                        in_=src[r0:r0 + rows].rearrange(
                            "(p r) d -> p (r d)", p=P, r=rpp))

                ab = abp.tile([P, 4 * w], f32, tag="ab")
                # One mul for both products: out elem (h, r, d, c) reads
                # xc[h*w + r*D + d] (dup over c) and G12[h*2D + 2d + c]
                # (dup over r). 5-D APs collapse to <=3 free dims in
                # lowering (out: 1, x: 2, gamma: 3).
                ab5 = ab[:].rearrange("p (h r d two) -> p h r d two",
                                      h=2, r=rpp, d=D, two=2)
                xdup = (xc[:].rearrange("p (h r d) -> p h r d",
                                        h=2, r=rpp, d=D)
                        .unsqueeze(4).broadcast_to([P, 2, rpp, D, 2]))
                gv = (gt[:].rearrange("p (h d two) -> p h d two",
                                      h=2, d=D, two=2)
                      .unsqueeze(2).broadcast_to([P, 2, rpp, D, 2]))
                nc.vector.tensor_mul(out=ab5, in0=xdup, in1=gv)
                # out = A + B, in place into the A half; store reads it.
                nc.vector.tensor_add(out=ab[:, :2 * w], in0=ab[:, :2 * w],
                                     in1=ab[:, 2 * w:])
                nc.scalar.dma_start(
                    out=out[r0:r0 + rows].rearrange("(p r) d -> p (r d)",
                                                    p=P, r=rpp),
                    in_=ab[:, :2 * w])
                r0 += rows
    nc.compile()
    return nc


def _get_program():
    if "nc" not in _CACHE:
        _CACHE["nc"] = _build_program()
    return _CACHE["nc"]


def _gamma_vector(gamma_real, gamma_imag):
    gr = np.asarray(gamma_real, dtype=np.float32)
    gi = np.asarray(gamma_imag, dtype=np.float32)
    g1 = np.stack([gr, gi], axis=-1).ravel()                 # [2*D]
    g2 = np.stack([-gi, gr], axis=-1).ravel()
    g12 = np.concatenate([g1, g2])                           # [4*D]
    return np.ascontiguousarray(np.broadcast_to(g12, (P, 4 * D)))


def _in_maps(x_real, x_imag, gamma_real, gamma_imag):
    g12 = _gamma_vector(gamma_real, gamma_imag)
    return [{
        "xr": np.ascontiguousarray(x_real[b], dtype=np.float32),
        "xi": np.ascontiguousarray(x_imag[b], dtype=np.float32),
        "g12": g12,
    } for b in range(N_CORES)]


def kernel(x_real, x_imag, gamma_real, gamma_imag):
    from concourse.bass_utils import run_bass_kernel_spmd

    nc = _get_program()
    res = run_bass_kernel_spmd(
        nc, _in_maps(x_real, x_imag, gamma_real, gamma_imag),
        list(range(N_CORES)))
    shards = [res.results[c]["out"].view(np.complex64) for c in range(N_CORES)]
    return np.stack(shards, axis=0)


def run_traced(x_real, x_imag, gamma_real, gamma_imag, **kw):
    """Profiled run (for test.py): returns BassKernelResults with
    exec_time_ns populated from the NTFF profile."""
    from concourse.bass_utils import run_bass_kernel_spmd

    nc = _get_program()
    return run_bass_kernel_spmd(
        nc, _in_maps(x_real, x_imag, gamma_real, gamma_imag),
        list(range(N_CORES)), trace=True, **kw)
